# revision 43
# baseline (speedup 1.0000x reference)
"""Multi-head attention TRN2 kernel, 8-core SPMD, v2.

Sharding: core c owns batch b=c//2 and head-group hg=c%2 (8 of 16 heads).
Each core projects Q/K/V for its 8 heads over the full 2048-token sequence
of its batch, runs attention, and computes a PARTIAL output projection
(contraction over its 512 features). The host sums the two partial outputs
per batch (the all-reduce of the tensor-parallel sharding) -- no on-device
collectives.

Dtypes / PE tricks (cost model: matmul time ~ moving-dim rows; fp8 DoubleRow
runs 0.5 cyc/row; zero-padded pair blocks let DR cover 64-dim score
contractions at half cost):
  - Q/K projections: fp8e4m3 DoubleRow (contraction 1024 = 4 x (128x2)).
  - Scores: fp8 DR, contraction (64 real + 64 zero-pad), out [128k, 1024q].
  - V projection + PV: bf16, feat-orientation PV with a ones column in V
    producing the softmax denominator as PSUM row 64.
  - Out projection: fp8 DR with head-pair i-blocks (both heads of a pair
    evict to partitions 0..63, different free blocks).
  - exp on scalar engine (scale=1/8, no max subtraction: scores/8 ~ N(0,1)),
    0/1 mask applied multiplicatively post-exp, split vector/pool engines.

All phases share one PSUM regime: [128,1024] f32 score-class tiles (2 bufs,
4 banks) + [65,1024] f32 PV accumulators (2 bufs, 4 banks). Projections,
scores and the out-projection all rotate through the score-class tag, so
phases pipeline without PSUM pool boundaries.
"""

import numpy as np

B, S, D, H, DH = 4, 2048, 1024, 16, 64
NCORES = 8
POOL_EVERY = 5  # every POOL_EVERY-th mask-mul chunk goes to the Pool engine

_CACHE = {}


def _build():
    from contextlib import ExitStack

    import concourse.mybir as mybir
    import concourse.tile as tile
    from concourse import bacc

    f32 = mybir.dt.float32
    bf16 = mybir.dt.bfloat16
    f8 = mybir.dt.float8e4
    EXP = mybir.ActivationFunctionType.Exp
    IDENT = mybir.ActivationFunctionType.Identity
    DR = mybir.MatmulPerfMode.DoubleRow

    nc = bacc.Bacc(
        "TRN2",
        target_bir_lowering=False,
        debug=False,
        enable_asserts=False,
        num_devices=NCORES,
    )

    xq8_d = nc.dram_tensor("xq8", [128, 4, 2, 2, 1024], bf16, kind="ExternalInput").ap()
    xk8_d = nc.dram_tensor("xk8", [128, 4, 2, 2, 1024], bf16, kind="ExternalInput").ap()
    xv_d = nc.dram_tensor("xv", [128, 16, 4, 2, 128], bf16, kind="ExternalInput").ap()
    wq8_d = nc.dram_tensor("wq8", [128, 4, 4, 2, 128], bf16, kind="ExternalInput").ap()
    wk8_d = nc.dram_tensor("wk8", [128, 4, 4, 2, 128], bf16, kind="ExternalInput").ap()
    wv_d = nc.dram_tensor("wv", [128, 4, 2, 2, 256], bf16, kind="ExternalInput").ap()
    wo8_d = nc.dram_tensor("wo8", [128, 4, 8, 128], bf16, kind="ExternalInput").ap()
    mk_d = nc.dram_tensor("mask_t", [128, 16, S], bf16, kind="ExternalInput").ap()
    wb_d = nc.dram_tensor("wb", [128, 16], f32, kind="ExternalInput").ap()
    out_d = nc.dram_tensor("out_t", [1024, S], bf16, kind="ExternalOutput").ap()
    dbg_q = nc.dram_tensor("dbg_q", [128, 2, 1024], bf16, kind="ExternalOutput").ap()
    dbg_k = nc.dram_tensor("dbg_k", [128, 16, 128], bf16, kind="ExternalOutput").ap()
    dbg_v = nc.dram_tensor("dbg_v", [128, 8, 65], bf16, kind="ExternalOutput").ap()
    dbg_x = nc.dram_tensor("dbg_x", [128, 4, 1024], bf16, kind="ExternalOutput").ap()

    with tile.TileContext(nc) as tc:
        stk = ExitStack()

        kpool = stk.enter_context(tc.tile_pool(name="konst", bufs=1))
        wb_sb = kpool.tile([128, 16], f32, name="wb_sb")
        wq8_sb = kpool.tile([128, 4, 4, 2, 128], bf16, name="wq8_sb")
        wk8_sb = kpool.tile([128, 4, 4, 2, 128], bf16, name="wk8_sb")
        wv_sb = kpool.tile([128, 4, 2, 2, 256], bf16, name="wv_sb")
        wo8_sb = kpool.tile([128, 4, 8, 128], bf16, name="wo8_sb")

        mpool = stk.enter_context(tc.tile_pool(name="msk", bufs=1))
        qkpool = stk.enter_context(tc.tile_pool(name="qk8", bufs=1))
        qp8 = [qkpool.tile([128, 2, 1024], bf16, name=f"qp8_{fb}") for fb in range(4)]
        kp8 = [qkpool.tile([128, 16, 128], bf16, name=f"kp8_{fb}") for fb in range(4)]
        vpool = stk.enter_context(tc.tile_pool(name="vsb", bufs=1))
        v_sb = [vpool.tile([128, 8, 65], bf16, name=f"v_{kc}") for kc in range(16)]
        xapool = stk.enter_context(tc.tile_pool(name="xatt", bufs=1))
        x_att8 = [
            xapool.tile([128, 4, 1024], bf16, name=f"x_att8_{th}")
            for th in range(2)
        ]
        hstage = [
            xapool.tile([64, 1024], bf16, name=f"hstage_{i}") for i in range(2)
        ]

        pepool = stk.enter_context(tc.tile_pool(name="pep", bufs=18))
        rpool = stk.enter_context(tc.tile_pool(name="rcp", bufs=1))
        bpool = stk.enter_context(tc.tile_pool(name="rbp", bufs=1))
        opool = stk.enter_context(tc.tile_pool(name="ost", bufs=2))
        xvpool = stk.enter_context(tc.tile_pool(name="xvs", bufs=2))

        xistk = ExitStack()
        xipool = xistk.enter_context(tc.tile_pool(name="xi8", bufs=1))

        stpool = stk.enter_context(tc.tile_pool(name="pst", bufs=2, space="PSUM"))
        xtpool = stk.enter_context(tc.tile_pool(name="pxt", bufs=2, space="PSUM"))

        # ---- input DMAs (SP queue order = need order) ------------------
        nc.sync.dma_start(wb_sb[:], wb_d[:, :])
        nc.sync.dma_start(wq8_sb[:], wq8_d[:, :, :, :, :])
        nc.scalar.dma_start(wk8_sb[:], wk8_d[:, :, :, :, :])
        nc.gpsimd.dma_start(wv_sb[:], wv_d[:, :, :, :, :])

        mk_t = []
        for kc in range(16):
            mt = mpool.tile([128, 1024], bf16, tag=f"mk{kc}", name=f"mk{kc}")
            deng = nc.scalar if kc < 8 else nc.sync
            deng.dma_start(mt[:], mk_d[:, kc, 0:1024])
            mk_t.append(mt)
        nc.sync.dma_start(wo8_sb[:], wo8_d[:, :, :, :])

        for kc in range(16):
            nc.vector.memset(v_sb[kc][:, :, 64:65], 1.0)

        # ---- projections ----------------------------------------------
        def proj_fb(xsrc, wsb, dst, bc0, fb, deng, qstyle=True):
            pss = [
                stpool.tile([128, 1024], f32, tag="st", name="pp")
                for _ in range(2)
            ]
            for ic in range(4):
                xt_i = xipool.tile(
                    [128, 2, 2, 1024], bf16, tag="xi", name="xi"
                )
                deng.dma_start(xt_i[:], xsrc[:, ic, :, :, :])
                for th in range(2):
                    for nh in range(2):
                        nsl = slice(nh * 512, (nh + 1) * 512)
                        for i in range(2):
                            nc.tensor.matmul(
                                pss[th][:, nsl],
                                lhsT=wsb[:, ic, fb, i, :],
                                rhs=xt_i[:, th, i, nsl],
                                start=(ic == 0 and i == 0),
                                stop=(ic == 3 and i == 1),
                            )
            for th in range(2):
                if qstyle:
                    nc.vector.tensor_scalar_add(
                        dst[fb][:, th, :],
                        pss[th][:],
                        wb_sb[:, bc0 + fb : bc0 + fb + 1],
                    )
                else:
                    nc.vector.tensor_scalar_add(
                        dst[fb][:, th * 8 : (th + 1) * 8, :],
                        pss[th][:].rearrange("p (a b) -> p a b", b=128),
                        wb_sb[:, bc0 + fb : bc0 + fb + 1],
                    )

        xv_cache = {}

        def vchain(vh, kc):
            kc2 = kc // 2
            if (vh, kc2) not in xv_cache:
                xv_t = xvpool.tile(
                    [128, 2, 4, 2, 128], bf16, tag="xv", name="xv_t"
                )
                nc.gpsimd.dma_start(
                    xv_t[:], xv_d[:, 2 * kc2 : 2 * kc2 + 2, :, :, :]
                )
                xv_cache[(vh, kc2)] = xv_t
            xv_t = xv_cache[(vh, kc2)]
            ps = stpool.tile([128, 1024], f32, tag="st", name="pv")
            for ici in range(8):
                ic, i = ici // 2, ici % 2
                nc.tensor.matmul(
                    ps[:, 0:256],
                    lhsT=xv_t[:, kc % 2, ic, i, :],
                    rhs=wv_sb[:, ic, vh, i, :],
                    start=(ici == 0),
                    stop=(ici == 7),
                )
            nc.vector.tensor_copy(
                v_sb[kc][:, 4 * vh : 4 * vh + 4, 0:64],
                ps[:, 0:256].rearrange("p (h f) -> p h f", f=64),
            )

        cc = [0]
        fillers = []

        def unit(h, qh, nfill=0):
            fb, j = h // 2, h % 2
            qsl = slice(qh * 1024, (qh + 1) * 1024)
            fill_at = {
                round((i + 1) * 16 / (nfill + 1)) - 1 for i in range(nfill)
            }
            pe_tiles = []
            for kc in range(16):
                if kc in fill_at and fillers:
                    fillers.pop(0)()
                st = stpool.tile([128, 1024], f32, tag="st", name="st")
                for nh in range(2):
                    nsl = slice(nh * 512, (nh + 1) * 512)
                    nc.tensor.matmul(
                        st[:, nsl],
                        lhsT=kp8[fb][64 * j : 64 * j + 64, kc, :],
                        rhs=qp8[fb][64 * j : 64 * j + 64, qh, nsl],
                        start=True,
                        stop=True,
                    )
                pe = pepool.tile([128, 1024], bf16, tag="pe", name="pe")
                nc.scalar.activation(pe[:], st[:], EXP, scale=0.125)
                eng = nc.vector if True else nc.gpsimd
                eng.tensor_mul(pe[:], pe[:], mk_t[kc][:])
                pe_tiles.append(pe)
                cc[0] += 1
            xt = xtpool.tile([65, 1024], f32, tag="xt", name="xt")
            for nh in range(2):
                nsl = slice(nh * 512, (nh + 1) * 512)
                for kc in range(16):
                    nc.tensor.matmul(
                        xt[:, nsl],
                        lhsT=v_sb[kc][:, h, :],
                        rhs=pe_tiles[kc][:, nsl],
                        start=(kc == 0),
                        stop=(kc == 15),
                    )
            rc = rpool.tile([1, 1024], f32, tag="rc", name="rc")
            nc.vector.reciprocal(rc[:], xt[64:65, :])
            rb = bpool.tile([64, 1024], f32, tag="rb", name="rb")
            nc.gpsimd.partition_broadcast(rb[:], rc[:])
            if j == 0:
                nc.vector.tensor_mul(
                    x_att8[qh][0:64, fb, :], xt[0:64, :], rb[:]
                )
            else:
                hs_t = hstage[qh]
                nc.vector.tensor_mul(hs_t[:], xt[0:64, :], rb[:])
                nc.sync.dma_start(x_att8[qh][64:128, fb, :], hs_t[:])

        def oproj(ofb, th):
            ps = stpool.tile([128, 1024], f32, tag="st", name="po")
            for nh in range(2):
                nsl = slice(nh * 512, (nh + 1) * 512)
                for ic in range(4):
                    nc.tensor.matmul(
                        ps[:, nsl],
                        lhsT=wo8_sb[:, ic, ofb, :],
                        rhs=x_att8[th][:, ic, nsl],
                        start=(ic == 0),
                        stop=(ic == 3),
                    )
            co = opool.tile([128, 1024], bf16, tag="co", name="co")
            if th == 1 and ofb % 2 == 0:
                nc.scalar.activation(
                    co[:], ps[:], IDENT, bias=wb_sb[:, 8 + ofb : 9 + ofb]
                )
            else:
                nc.vector.tensor_scalar_add(
                    co[:], ps[:], wb_sb[:, 8 + ofb : 9 + ofb]
                )
            deng = nc.sync if ofb % 2 == 0 else nc.scalar
            deng.dma_start(
                out_d[ofb * 128 : (ofb + 1) * 128, th * 1024 : (th + 1) * 1024],
                co[:],
            )

        # prologue: fb0 Q/K projections only; everything else drip-fed
        proj_fb(xq8_d, wq8_sb, qp8, 0, 0, nc.sync)
        proj_fb(xk8_d, wk8_sb, kp8, 4, 0, nc.scalar, qstyle=False)
        for kc in range(16):
            vchain(0, kc)
        for fb in range(1, 4):
            proj_fb(xq8_d, wq8_sb, qp8, 0, fb, nc.sync)
            proj_fb(xk8_d, wk8_sb, kp8, 4, fb, nc.scalar, qstyle=False)
        for kc in range(16):
            vchain(1, kc)
        xistk.close()
        for h in range(8):
            unit(h, 0)

        # qh=1 mask refill (WAR on last qh=0 reader per chunk)
        for kc in range(16):
            mt = mpool.tile([128, 1024], bf16, tag=f"mk{kc}", name=f"mk{kc}b")
            nc.sync.dma_start(mt[:], mk_d[:, kc, 1024:2048])
            mk_t[kc] = mt

        for h in range(8):
            unit(h, 1)
            oproj(h, 0)
        for ofb in range(8):
            oproj(ofb, 1)
        nc.sync.dma_start(dbg_q[:, :, :], qp8[3][:])
        nc.sync.dma_start(dbg_k[:, :, :], kp8[3][:])
        nc.sync.dma_start(dbg_v[:, :, :], v_sb[0][:])
        nc.sync.dma_start(dbg_x[:, :, :], x_att8[0][:])
        stk.close()

    nc.compile()
    return nc


def _get_nc():
    if "nc" not in _CACHE:
        _CACHE["nc"] = _build()
    return _CACHE["nc"]


def _prep(query, key, value, mask, Wq, bq, Wk, bk, Wv, bv, Wo, bo):
    import ml_dtypes

    f = np.float32
    f8 = ml_dtypes.float8_e4m3fn
    b16 = ml_dtypes.bfloat16

    def x8(x2d):  # [2048 t, 1024 d] -> [128 p, 4 ic, 2 th, 2 i, 1024 t] fp8
        xt = np.ascontiguousarray(np.asarray(x2d, f).T)  # [1024 d, 2048]
        a = xt.reshape(4, 2, 128, 2, 1024)  # ic, i, p, th, t
        return np.ascontiguousarray(a.transpose(2, 0, 3, 1, 4)).astype(b16)

    def w8(Ws):  # [512 f, 1024 d] -> [128 p, 4 ic, 4 fb, 2 i, 128 f] fp8
        wt = np.ascontiguousarray(np.asarray(Ws, f).T)  # [1024 d, 512 f]
        return np.ascontiguousarray(
            wt.reshape(4, 2, 128, 4, 128).transpose(2, 0, 3, 1, 4)
        ).astype(b16)

    m2 = np.asarray(mask)[0, 0]  # [Sq, Sk]
    mask_t = np.ascontiguousarray(
        np.ascontiguousarray(m2.T).reshape(16, 128, S).transpose(1, 0, 2)
    ).astype(b16)

    Wq, Wk, Wv, Wo = (np.asarray(a, f) for a in (Wq, Wk, Wv, Wo))
    bq, bk, bv, bo = (np.asarray(a, f) for a in (bq, bk, bv, bo))
    bo_eff = (
        np.asarray(bo, np.float64)
        + np.asarray(Wo, np.float64) @ np.asarray(bv, np.float64)
    ).astype(f)

    in_maps = []
    for c in range(NCORES):
        b, hg = c // 2, c % 2
        hs = hg * 512
        wvs = Wv[hs : hs + 512, :]  # [512 f, 1024 d]
        wv_t = np.ascontiguousarray(
            wvs.T.reshape(4, 2, 128, 2, 256).transpose(2, 0, 3, 1, 4)
        ).astype(b16)
        wos = np.ascontiguousarray(Wo.T[hs : hs + 512, :])  # [512 d, 1024 o]
        wo8 = np.ascontiguousarray(
            wos.reshape(4, 128, 8, 128).transpose(1, 0, 2, 3)
        ).astype(b16)
        vt = np.ascontiguousarray(np.asarray(value)[b].astype(f).T)  # [1024, 2048]
        xv_t = np.ascontiguousarray(
            vt.reshape(4, 2, 128, 16, 128).transpose(2, 3, 0, 1, 4)
        ).astype(b16)
        wb = np.zeros((128, 16), f)
        wb[:, 0:4] = bq[hs : hs + 512].reshape(4, 128).T
        wb[:, 4:8] = bk[hs : hs + 512].reshape(4, 128).T
        if hg == 0:
            wb[:, 8:16] = bo_eff.reshape(8, 128).T
        in_maps.append(
            {
                "xq8": x8(np.asarray(query)[b]),
                "xk8": x8(np.asarray(key)[b]),
                "xv": xv_t,
                "wq8": w8(Wq[hs : hs + 512, :]),
                "wk8": w8(Wk[hs : hs + 512, :]),
                "wv": wv_t,
                "wo8": wo8,
                "mask_t": mask_t,
                "wb": np.ascontiguousarray(wb),
            }
        )
    return in_maps


def kernel(**inputs):
    from concourse.bass_utils import run_bass_kernel_spmd

    np_inputs = {k: np.asarray(v) for k, v in inputs.items()}
    in_maps = _prep(**np_inputs)
    nc = _get_nc()
    res = run_bass_kernel_spmd(nc, in_maps, list(range(NCORES)))
    out = np.empty((B, S, D), np.float32)
    for b in range(B):
        p0 = res.results[2 * b]["out_t"].astype(np.float32)
        p1 = res.results[2 * b + 1]["out_t"].astype(np.float32)
        out[b] = (p0 + p1).T
    return out
